# revision 47
# baseline (speedup 1.0000x reference)
"""Trainium2 Bass kernel for nn_CircuitLoss (classwise pairwise cossim + binary entropy).

Math notes
----------
The reference builds S = N @ N.T ([B,B]) with N = row-normalized activations and
reduces S @ M against the one-hot membership M.  Algebraically
    mSm[k]      = sum_{i,j in class k} S_ij = || sum_{i in k} N_i ||^2
    sum_diag[k] = sum_{i in k} ||N_i||^2
so the whole gram matrix collapses into a [K, D] class-sum  Csum = (M * recip).T @ acts
(recip folded into the membership matrix so raw activation rows never need a
normalize pass).  Each core processes B/8 = 1024 rows and ships back:
    csum  [2, 64, 4096]  per-tensor class sums
    small [128, 64]      per row-tile sqsum / recip / entropy partial sums
The host does the tiny O(K*D + B) finalization.

Binary entropy: H(p) = -(p*ln(p_clip) + (1-p)*ln(1-p)).  The lower clip is folded
into the activation bias (ln(p + 1e-8) == ln(max(p,1e-8)) for p<=~0; the bias
shifts the mean by ~1e-8, far below tolerance).  Per tile:
    v = Ln(p + 1e-8)              (ACT, bf16 out)
    u = Ln(-p + 1), accum -> usum (ACT, bf16 out)
    t = v - u                     (DVE tensor_tensor, bf16, 2x mode)
    w = p * t,   accum -> wsum    (DVE tensor_tensor_reduce)
    sum H = -(sum w + sum u)
Row sum-of-squares for normalization: ACT Square+accum for tensor 0, DVE
tensor_tensor_reduce for tensor 1 (engine balance).  rsqrt = Exp(-0.5*Ln(ss))
keeps every activation inside the natural_log_exp_and_others table set (zero
table switches; the Sqrt/Rsqrt funcs live in other sets / are banned).
"""

import os

os.environ.setdefault("MYCRO_LOCAL_CACHE", "1")

import numpy as np

import concourse.bass as bass
import concourse.bacc as bacc
import concourse.mybir as mybir
from concourse.bass_utils import run_bass_kernel_spmd
from concourse.tile import TileContext

B, D, K = 8192, 4096, 64
NCORES = 8
RPC = B // NCORES  # rows per core
NT = RPC // 128    # 128-row tiles per core
EPS = 1e-8
LAMBDA_SIM = 1.0
LAMBDA_SPARSITY = 0.001

F32 = mybir.dt.float32
F32R = mybir.dt.float32r
BF16 = mybir.dt.bfloat16
AF = mybir.ActivationFunctionType
ALU = mybir.AluOpType

# small[q] layout (each [128, 16, 8] f32, value at [:, slot, 0], 32B-padded slots):
# q=0: ss for t=0 (slots 0..7, ACT-written)   q=1: ss for t=1 (slots 0..7, DVE)
# q=2: recip (slot t*8+i, DVE)  q=3: usum (slot t*8+i, ACT)  q=4: wsum (slot t*8+i, DVE)
_QSS0, _QSS1, _QRC, _QUS, _QWS = 0, 1, 2, 3, 4

_CACHE = {}
LAST_RESULT = None  # BassKernelResults of the most recent run (for profiling)


def _register_const(nc, dtype, value):
    t = nc.alloc_sbuf_tensor(f"const-{dtype.name}-{value}", [128, 1], dtype)
    nc.gpsimd.memset(t.ap(), value)
    nc.const_aps.aps[(dtype, value)] = t.ap()


def _build(reps=1):
    nc = bacc.Bacc(trn_type="TRN2")
    _register_const(nc, F32, EPS)
    nc.all_engine_barrier()

    a1 = nc.dram_tensor("acts1b", [RPC, D], F32, kind="ExternalInput").ap()
    a8 = nc.dram_tensor("acts8b", [RPC, D], F32, kind="ExternalInput").ap()
    m1 = nc.dram_tensor("mask1b", [RPC, D], F32, kind="ExternalInput").ap()
    m8 = nc.dram_tensor("mask8b", [RPC, D], F32, kind="ExternalInput").ap()
    mm = nc.dram_tensor("memb", [RPC, K], F32, kind="ExternalInput").ap()
    csum = nc.dram_tensor("csum", [2, K, D], F32, kind="ExternalOutput").ap()
    small = nc.dram_tensor("small", [5, 128, 16, 8], F32, kind="ExternalOutput").ap()

    acts_d = (a1, a8)
    mask_d = (m1, m8)

    with TileContext(nc) as tc:
        with (
            tc.tile_pool(name="io", bufs=2) as io_pool,
            tc.tile_pool(name="bf", bufs=2) as bf_pool,
            tc.tile_pool(name="aux", bufs=2) as aux_pool,
            tc.tile_pool(name="ps", bufs=1, space="PSUM") as ps_pool,
        ):
            ss0_sb = aux_pool.tile([128, 16, 8], F32, tag="ss0_sb", bufs=1)
            ss1_sb = aux_pool.tile([128, 16, 8], F32, tag="ss1_sb", bufs=1)
            rc_sb = aux_pool.tile([128, 16, 8], F32, tag="rc_sb", bufs=1)
            us_sb = aux_pool.tile([128, 16, 8], F32, tag="us_sb", bufs=1)
            ws_sb = aux_pool.tile([128, 16, 8], F32, tag="ws_sb", bufs=1)
            m_all = aux_pool.tile([128, NT, K], F32, tag="m_all", bufs=1)
            nc.sync.dma_start(m_all, mm.rearrange("(i p) k -> p i k", p=128))
            for tile_sb in (ss0_sb, ss1_sb, rc_sb, us_sb, ws_sb):
                nc.vector.memset(tile_sb, 0.0)

            # Pre-load the one activation table set covering Ln/Exp/Square/Copy
            # (natural_log_exp_and_others, id 6) so the act-table pass doesn't
            # thrash between per-function sets (33 loads -> 1).
            nc.scalar.add_instruction(
                mybir.InstLoadActFuncSet(
                    name=nc.get_next_instruction_name(), act_func_set_id=6,
                    ins=[], outs=[],
                )
            )

            for rep in range(reps):
              for t in range(2):
                a_t = acts_d[t].rearrange("(i p) d -> i p d", p=128)
                m_t = mask_d[t].rearrange("(i p) d -> i p d", p=128)
                ps = [
                    ps_pool.tile([64, 512], F32, tag=f"ps{c}", bufs=1, name=f"ps{t}_{c}")
                    for c in range(8)
                ]
                # Mask DMAs are issued a few tiles ahead of acts: both streams
                # share the SDMA engines ~fairly while queued, so the mask
                # stream (whose consumer chain ACT v->u->DVE t->w is the
                # longest) drains first and the post-stream tail shortens.
                mask_tiles = []
                for i in range(min(3, NT)):
                    mk = io_pool.tile([128, D], F32, tag="mask", bufs=4, name=f"mask{t}_{i}")
                    nc.scalar.dma_start(mk, m_t[i])
                    mask_tiles.append(mk)
                for i in range(NT):
                    last_tile = t == 1 and i == NT - 1
                    # SWDGE cast-DMA rounds f32 -> f32r during the load; the
                    # PE streams f32r at full (1 cyc/row) rate vs 4 for f32.
                    acts = io_pool.tile([128, D], F32R, tag="acts", bufs=3, name=f"acts{t}_{i}")
                    if last_tile:
                        # halve the final load so its consumers start ~3us earlier
                        nc.gpsimd.dma_start(acts[:, : D // 2], a_t[i][:, : D // 2])
                        nc.gpsimd.dma_start(acts[:, D // 2 :], a_t[i][:, D // 2 :])
                    else:
                        nc.gpsimd.dma_start(acts, a_t[i])
                    if i + 3 < NT:
                        mk = io_pool.tile([128, D], F32, tag="mask", bufs=4, name=f"mask{t}_{i+3}")
                        nc.scalar.dma_start(mk, m_t[i + 3])
                        mask_tiles.append(mk)
                    mask = mask_tiles[i]

                    sst = ss0_sb if t == 0 else ss1_sb
                    ss_col = sst[:, i, 0:1]
                    sq = bf_pool.tile([128, D], BF16, tag="scr", bufs=2, name=f"sq{t}_{i}")
                    if last_tile:
                        # parallel half-sums on ACT + DVE, combined below
                        nc.scalar.activation(sq[:, : D // 2], acts[:, : D // 2],
                                             AF.Square, accum_out=sst[:, i, 1:2])
                        nc.vector.scalar_tensor_tensor(
                            out=sq[:, D // 2 :], in0=acts[:, D // 2 :], scalar=0.0,
                            in1=acts[:, D // 2 :], op0=ALU.bypass, op1=ALU.mult,
                            accum_out=sst[:, i, 2:3],
                        )
                        nc.vector.tensor_add(ss_col, sst[:, i, 1:2], sst[:, i, 2:3])
                    elif t == 0:
                        nc.scalar.activation(sq, acts, AF.Square, accum_out=ss_col)
                    else:
                        nc.vector.scalar_tensor_tensor(
                            out=sq, in0=acts, scalar=0.0, in1=acts,
                            op0=ALU.bypass, op1=ALU.mult, accum_out=ss_col,
                        )

                    # recip = min(exp(-0.5*ln(ss)), 1e8)  ==  1 / max(sqrt(ss), eps)
                    lnss = aux_pool.tile([128, 1], F32, tag="tiny", bufs=4, name=f"lnss{t}_{i}")
                    nc.scalar.activation(lnss, ss_col, AF.Ln)
                    rs = aux_pool.tile([128, 1], F32, tag="tiny2", bufs=4, name=f"rs{t}_{i}")
                    nc.scalar.activation(rs, lnss, AF.Exp, scale=-0.5)
                    rc_col = rc_sb[:, t * 8 + i, 0:1]
                    nc.vector.tensor_scalar_min(rc_col, rs, 1e8)

                    mp = aux_pool.tile([128, K], F32R, tag="mp", bufs=3, name=f"mp{t}_{i}")
                    nc.vector.tensor_scalar_mul(mp, m_all[:, i, :], rc_col)

                    for c in range(8):
                        nc.tensor.matmul(
                            ps[c][:, :],
                            lhsT=mp,
                            rhs=acts[:, c * 512 : (c + 1) * 512],
                            start=(i == 0),
                            stop=(i == NT - 1),
                        )

                    # Last tile of the last phase: split in halves so the
                    # post-final-DMA dependency chain (v->u->t->w) is ~2x shorter.
                    slot = t * 8 + i
                    if t == 1 and i == NT - 1:
                        halves = ((0, D // 2), (D // 2, D))
                    else:
                        halves = ((0, D),)
                    for h, (c0, c1) in enumerate(halves):
                        mh = mask[:, c0:c1]
                        hw = c1 - c0
                        v = bf_pool.tile([128, hw], BF16, tag="v", bufs=2, name=f"v{t}_{i}_{h}")
                        nc.scalar.activation(v, mh, AF.Ln, bias=EPS)
                        u = bf_pool.tile([128, hw], BF16, tag="u", bufs=2, name=f"u{t}_{i}_{h}")
                        nc.scalar.activation(u, mh, AF.Ln, scale=-1.0, bias=1.0,
                                             accum_out=us_sb[:, slot, h : h + 1])
                        tvu = bf_pool.tile([128, hw], BF16, tag="tvu", bufs=2, name=f"tvu{t}_{i}_{h}")
                        nc.vector.tensor_sub(tvu, v, u)
                        w = bf_pool.tile([128, hw], BF16, tag="scr", bufs=2, name=f"w{t}_{i}_{h}")
                        nc.vector.scalar_tensor_tensor(
                            out=w, in0=mh, scalar=0.0, in1=tvu,
                            op0=ALU.bypass, op1=ALU.mult,
                            accum_out=ws_sb[:, slot, h : h + 1],
                        )

                stage = aux_pool.tile([64, D], F32, tag="stage", bufs=1, name=f"stage{t}")
                for c in range(8):
                    if t == 1:
                        nc.scalar.copy(stage[:, c * 512 : (c + 1) * 512], ps[c][:, :])
                    else:
                        nc.vector.tensor_copy(stage[:, c * 512 : (c + 1) * 512], ps[c][:, :])
                    if c == 3:
                        nc.sync.dma_start(csum[t][:, : D // 2], stage[:, : D // 2])
                nc.sync.dma_start(csum[t][:, D // 2 :], stage[:, D // 2 :])

            for q, tile_sb in enumerate((ss0_sb, ss1_sb, rc_sb, us_sb, ws_sb)):
                nc.sync.dma_start(small[q], tile_sb)
    nc.compile()
    return nc


def _get_nc():
    if "nc" not in _CACHE:
        _CACHE["nc"] = _build()
    return _CACHE["nc"]


def _finalize(memb_f32, csums, smalls):
    """Host-side O(B + K*D) reduction. csums: [NCORES][2,K,D], smalls: [NCORES][128,64]."""
    lam_sim, lam_sp = LAMBDA_SIM, LAMBDA_SPARSITY
    ncores = len(csums)
    b_eff = memb_f32.shape[0]
    n_per_class = memb_f32.sum(axis=0).astype(np.float64)  # [K]

    outs = []
    for t in range(2):
        csum_t = np.zeros((K, D), np.float64)
        for c in range(ncores):
            csum_t += csums[c][t].astype(np.float64)
        mSm = (csum_t * csum_t).sum(axis=1)  # [K]

        # diag[g] = ss[g] * recip[g]^2, summed per class
        diag = np.empty(b_eff, np.float64)
        for c in range(ncores):
            s = smalls[c]  # [5, 128, 16, 8]
            ss = s[_QSS0 + t, :, 0:8, 0].astype(np.float64)    # [128, 8] (p, i)
            rc = s[_QRC, :, t * 8 : t * 8 + 8, 0].astype(np.float64)
            d = ss * rc * rc                                    # [p, i]
            # global row g = c*RPC + i*128 + p
            diag[c * RPC : (c + 1) * RPC] = d.T.reshape(-1)
        sum_diag = memb_f32.T.astype(np.float64) @ diag  # [K]

        pair_sum = 0.5 * (mSm - sum_diag)
        n_pairs = 0.5 * n_per_class * (n_per_class - 1.0)
        valid = n_per_class >= 2.0
        per_class = np.where(valid, pair_sum / np.maximum(n_pairs, 1.0), 0.0)
        n_valid = valid.sum()
        cossim = per_class.sum() / max(n_valid, 1.0) if n_valid > 0 else 0.0
        sim_loss = -cossim

        h_sum = 0.0
        for c in range(ncores):
            s = smalls[c].astype(np.float64)
            # sum all 8 lanes: split tiles use lanes 0/1, memset zeros elsewhere
            h_sum -= s[_QUS, :, t * 8 : t * 8 + 8, :].sum()
            h_sum -= s[_QWS, :, t * 8 : t * 8 + 8, :].sum()
        sp_loss = h_sum / (b_eff * D)
        outs.append((sim_loss, sp_loss))

    (sim1, sp1), (sim8, sp8) = outs
    total = (lam_sim * sim1 + lam_sp * sp1) + (lam_sim * sim8 + lam_sp * sp8)
    return np.array([total, sim1, sim8, sp1, sp8], dtype=np.float32)


def kernel(hard_class_probs, masked_activations_1b, masked_activations_8b, mask_1b, mask_8b):
    global LAST_RESULT
    hcp = np.asarray(hard_class_probs, np.float32)
    a1 = np.asarray(masked_activations_1b, np.float32)
    a8 = np.asarray(masked_activations_8b, np.float32)
    p1 = np.asarray(mask_1b, np.float32)
    p8 = np.asarray(mask_8b, np.float32)
    memb = (hcp > 0.5).astype(np.float32)

    nc = _get_nc()
    in_maps = []
    for c in range(NCORES):
        sl = slice(c * RPC, (c + 1) * RPC)
        in_maps.append({
            "acts1b": np.ascontiguousarray(a1[sl]),
            "acts8b": np.ascontiguousarray(a8[sl]),
            "mask1b": np.ascontiguousarray(p1[sl]),
            "mask8b": np.ascontiguousarray(p8[sl]),
            "memb": np.ascontiguousarray(memb[sl]),
        })

    trace_cores = None
    if os.environ.get("KERNEL_TRACE_CORES") == "all":
        trace_cores = list(range(NCORES))
    res = run_bass_kernel_spmd(
        nc, in_maps, core_ids=list(range(NCORES)), trace_cores=trace_cores
    )
    LAST_RESULT = res
    csums = [r["csum"] for r in res.results]
    smalls = [r["small"] for r in res.results]
    return _finalize(memb, csums, smalls)
